# revision 52
# baseline (speedup 1.0000x reference)
"""Cluster-attention GNN kernel for TRN2 (8 NeuronCores, SPMD data-parallel over graphs).

Math (per graph g of exactly 50 nodes, clusters c in 0..7):
  cnt[g,c]   = #nodes with cls==c
  ratio[g,c] = cnt[g,c] / sum_c cnt[g,c]^2
  newx[g,c]  = ratio[g,c] * sum_{nodes in (g,c)} x          (fold ratio per-node into one-hot)
  h = leaky_relu(newx @ W1 + b1, 0.45); a = h @ W2          (only G*K distinct rows!)
  w[g,c] = exp(a-m_g) / (1e-16 + sum_c cnt[g,c] * exp(a-m_g))   (b2 cancels in softmax)
  out[node] = w[g(node), cls(node)]

Device layout: shard = 1250 graphs/core, superblocks of P<=128 graphs.
Node-tile t of a superblock = graph pair (2t, 2t+1): partitions 0:50 and
64:114. One f32r matmul per tile (single-pass bf16x2 weight load, 3x fp32):
lhsT = x_tile[114,128], rhs = scaled one-hot [114,16] -> psum[:, 16t:16t+16]
accumulates new_x^T feature-major. The MLP runs on 512-column groups in bf16.
x loads are 2 DMAs per 64 pairs (3.2MB each) so per-DMA HWDGE/SEQ overhead
(~1.2us serialized) is negligible. Emission is phase-major software-pipelined
(pre-pass / matmul+MLP / softmax+store with lookahead 3): per-engine queues
are in-order, so independent superblocks' work is interleaved to keep every
queue draining at throughput instead of serializing each superblock's long
cross-engine latency chain. Segment reductions are free-dim DVE broadcasts in
graph-major [P, *] layout; graph-major <-> node layouts bridged by PE
transposes. Relative error ~3e-4 (f32r+bf16) vs the 2e-2 gate.
"""

import os
import sys
from contextlib import ExitStack

import numpy as np

import concourse.bass as bass
import concourse.tile as tile
import concourse.tile_sem_assignment as _tsa
from concourse import mybir
from concourse.bass_utils import run_bass_kernel_spmd

# _split_waits() below hoists extra semaphore waits onto NoOps, so the
# default 8 HWDGE completion lanes (finer-grained DMA tracking, better
# overlap) are fine even where an instruction depends on several DMAs.
_ = _tsa  # imported for potential lane tuning; default NUM_HWDGE_SEMS kept

# CoreSim's race detector requires every instruction to carry an update, but
# walrus's NOP encoding shares one semaphore_value field between wait and
# update and rejects a NOP carrying both. Sim runs set this True.
SIM_NOP_UPDATES = False


def _split_waits(nc):
    """Hoist all-but-one sync wait off each instruction onto NoOps before it.

    This walrus build's TPB instruction encodings have a single EVENTS slot,
    so codegen rejects any instruction carrying more than one semaphore wait
    ("Too many sync wait commands"). A NoOp on the same engine stalls that
    sequencer identically. The NoOp's zero-increment dummy update targets the
    split instruction's own update sem (same writer domain, so the race
    detector stays happy) without advancing it.
    """
    # engine -> a semaphore that engine already updates (same writer domain);
    # fall back to a DMA completion-lane sem (multi-writer by design)
    eng_sem: dict = {}
    dma_sem = None
    for blk in nc.m.functions[0].blocks:
        for ins in blk.instructions:
            si = ins.sync_info
            if not si or not si.on_update:
                continue
            u = si.on_update[0]
            if dma_sem is None and u.ant_name.startswith("DMAHW"):
                dma_sem = u
            if ins.engine not in eng_sem and (
                u.ant_name.split("_")[0] == str(ins.engine).split(".")[-1]
            ):
                eng_sem[ins.engine] = u

    k = 0
    for blk in nc.m.functions[0].blocks:
        new = []
        for ins in blk.instructions:
            si = ins.sync_info
            if si and len(si.on_wait) > 1:
                own = (
                    si.on_update[0]
                    if si.on_update
                    else eng_sem.get(ins.engine) or dma_sem
                )
                if own is None:
                    new.append(ins)
                    continue
                for w in si.on_wait[:-1]:
                    nop = mybir.InstNoOp(name=f"split-wait-{k}", ins=[], outs=[])
                    k += 1
                    nop.engine = ins.engine
                    upds = []
                    if SIM_NOP_UPDATES:
                        upds.append(
                            mybir.SyncUpdate(
                                sync_type="semaphore", id=own.id,
                                ant_name=own.ant_name, update_mode="sem-add-imm",
                                update_value=0, update_reg=None,
                            )
                        )
                    nop.sync_info = mybir.SyncInfo(on_wait=[w], on_update=upds)
                    new.append(nop)
                ins.sync_info = mybir.SyncInfo(
                    on_wait=[si.on_wait[-1]], on_update=si.on_update
                )
            new.append(ins)
        blk.instructions = new

F = 128          # feature dim
K = 8            # clusters per graph
NPG = 50         # nodes per graph
NEG_SLOPE = 0.45
N_CORES = 8
SKIP_X_DMA = False   # benchmarking aid: elide x loads (results garbage)

N_TOTAL = 500_000
G_TOTAL = 10_000
G_CORE = G_TOTAL // N_CORES        # 1250 graphs per core
N_CORE = N_TOTAL // N_CORES        # 62500 nodes per core

AluOp = mybir.AluOpType
Act = mybir.ActivationFunctionType
f32 = mybir.dt.float32
f32r = mybir.dt.float32r
bf16 = mybir.dt.bfloat16
i32 = mybir.dt.int32


def superblock_sizes(g_core: int) -> list[tuple[int, int]]:
    """Split g_core graphs into superblocks (P_pad, P_real) with P_pad in {64, 128}.

    P_pad must be 64 or 128 so the graph-major->node-layout PE transposes read
    at base partitions {0, 32, 64} (hardware row-group constraint). Trailing
    graphs are padded with dummy graphs (cls=0, x=0) up to P_pad.
    """
    sizes = []
    g = g_core
    while g > 0:
        if g >= 128:
            sizes.append((128, 128))
            g -= 128
        elif g > 64:
            # one padded 128-superblock beats two half-width ones: each
            # superblock pays a fixed pre/transpose/softmax chain cost
            sizes.append((128, g))
            g = 0
        else:
            sizes.append((64, g))
            g = 0
    return sizes


def build_kernel(g_core: int, reps: int = 1):
    """Build the per-core Bass program for a shard of g_core graphs.

    reps>1 repeats the whole computation in one NEFF (same inputs/outputs,
    idempotent) — used only for benchmarking: the marginal time slope over
    reps cancels the ~600us per-dispatch overhead of the axon PJRT path.
    """
    nc = bass.Bass("TRN2", target_bir_lowering=False, debug=False)

    n_core = g_core * NPG
    x_d = nc.dram_tensor("x", [n_core, F], f32r, kind="ExternalInput").ap()
    cls_d = nc.dram_tensor("cls", [g_core, NPG], i32, kind="ExternalInput").ap()
    # W1 | b1 | W2 packed into one [F, F+2] operand (fewer PJRT operands =
    # less per-dispatch overhead); fixed lookup tables are NEFF-embedded
    # Const tensors, not runtime inputs.
    wpk_d = nc.dram_tensor("wpk", [F, F + 2], bf16, kind="ExternalInput").ap()
    consts = make_const_inputs()
    ident_d = nc.inline_tensor(consts["ident"], name="identc").ap()
    iota16_d = nc.inline_tensor(consts["iota16"], name="iota16c").ap()
    iotak_d = nc.inline_tensor(consts["iotak"], name="iotakc").ap()
    out_d = nc.dram_tensor("out", [g_core, NPG], f32, kind="ExternalOutput").ap()

    sbs = superblock_sizes(g_core)

    with tile.TileContext(nc) as tc, ExitStack() as ctx:
        const = ctx.enter_context(tc.tile_pool(name="const", bufs=1))
        p1s = ctx.enter_context(tc.tile_pool(name="p1s", bufs=1))
        pre = ctx.enter_context(tc.tile_pool(name="pre", bufs=3))
        nodep = ctx.enter_context(tc.tile_pool(name="nodep", bufs=3))
        mlp = ctx.enter_context(tc.tile_pool(name="mlp", bufs=3))
        post = ctx.enter_context(tc.tile_pool(name="post", bufs=3))
        ps_nx = ctx.enter_context(tc.tile_pool(name="ps_nx", bufs=2, space="PSUM"))
        ps_h = ctx.enter_context(tc.tile_pool(name="ps_h", bufs=2, space="PSUM"))
        ps_a = ctx.enter_context(tc.tile_pool(name="ps_a", bufs=2, space="PSUM"))
        ps_tr = ctx.enter_context(tc.tile_pool(name="ps_tr", bufs=2, space="PSUM"))

        # --- constants ---
        wpk_sb = const.tile([F, F + 2], bf16, tag="wpk")
        nc.sync.dma_start(wpk_sb[:, :], wpk_d)
        w1_sb = wpk_sb[:, 0:F]
        b1_sb = wpk_sb[:, F : F + 1]
        w2_sb = wpk_sb[:, F + 1 : F + 2]
        ident = const.tile([F, F], f32, tag="ident")
        nc.sync.dma_start(ident[:, :], ident_d)
        # iota16[p, s] = s - seg_offset(p): rows 0:50 -> s (graph A), rows
        # 64:114 -> s-8 (graph B), junk rows -> s-64 (never equals a cls in
        # 0..7, so the one-hot is 0 there)
        iota16 = const.tile([F, 2 * K], f32, tag="iota16")
        nc.sync.dma_start(iota16[:, :], iota16_d)
        iotak = const.tile([F, K], f32, tag="iotak")  # row = [0..7] everywhere
        nc.sync.dma_start(iotak[:, :], iotak_d)

        # Node-layout x tiles: graph pair (2t, 2t+1) at partitions {0:50,
        # 64:114} (A, B). One DMA per parity loads XB_PAIRS pairs (one MLP
        # group): fewer, bigger DMAs so the ~1.2us serialized per-DMA
        # HWDGE+SEQ overhead amortizes over 800KB instead of 200KB. Junk
        # partitions 50:64 zeroed once (matmul reads 0:114).
        XB_PAIRS = 64
        N_XT = 2
        # PE weight loads are 3x faster for float32r (single-pass bf16x2)
        # than float32, and f32r is byte-identical to f32 — so x and the
        # one-hot run the segment matmuls in f32r with zero conversion ops.
        xts = [
            const.tile([F, XB_PAIRS * F], f32r, tag=f"xt{i}", name=f"xt{i}")
            for i in range(N_XT)
        ]
        for t in xts:
            # junk partitions: one-time zero on the Pool engine so matmul
            # reads of rows 50:64 never see non-finite bits (f32 bitcast:
            # Pool memset has no f32r encoding; zero bits are valid f32r)
            nc.gpsimd.memset(t[32:64, :].bitcast(f32), 0.0)
        xt_idx = 0
        x4 = x_d.rearrange(
            "(gp two n) f -> gp two n f", two=2, n=NPG
        )  # [pairs, 2, 50, 128]

      # Phase-major emission: per-engine queues are in-order, so emitting
      # sb0's whole chain before sb1's serializes the long cross-engine
      # latency chain 10x. Instead: phase 1 (pre-pass) for ALL superblocks,
      # then phase 2 (x loads + matmuls + MLP), then phase 3 (softmax +
      # store). Within a phase, consecutive superblocks are independent, so
      # every queue drains at engine throughput and phases overlap naturally
      # through the queues.
        sb_g0 = []
        g0 = 0
        for P, P_real in sbs:
            sb_g0.append(g0)
            g0 += P_real
        n_pairs_real = (g0 + 1) // 2  # real graph pairs (g_core/2)

        p1 = {}

        def emit_p1(si):
            P, P_real = sbs[si]
            g0 = sb_g0[si]
            half = P // 2
            segs = 2 * K * half  # one-hot columns in this superblock

            cls_i = pre.tile([128, NPG], i32, tag="cls_i")
            nc.sync.dma_start(cls_i[0:P_real, :], cls_d[g0 : g0 + P_real, :])
            cls_f = pre.tile([128, NPG], f32, tag="cls_f")
            if P_real < P:
                nc.vector.memset(cls_f[0:P, :], 0.0)  # pad graphs: all cluster 0
            nc.vector.tensor_copy(cls_f[0:P_real, :], cls_i[0:P_real, :])

            # masks[g, c, j] = (cls[g, j] == c), one broadcast is_equal
            masks = p1s.tile([128, K * NPG], bf16, tag=f"masks{si}")
            masks_v = masks[0:P, :].rearrange("p (c j) -> p c j", j=NPG)
            nc.vector.tensor_tensor(
                masks_v,
                iotak[0:P, :, None].broadcast_to([P, K, NPG]),
                cls_f[0:P, None, :].broadcast_to([P, K, NPG]),
                AluOp.is_equal,
            )
            cnt = p1s.tile([128, K], f32, tag=f"cnt{si}")
            nc.vector.reduce_sum(
                cnt[0:P, :, None], masks_v, axis=mybir.AxisListType.X
            )
            sq = pre.tile([128, K], f32, tag="sq")
            nc.vector.tensor_tensor(sq[0:P, :], cnt[0:P, :], cnt[0:P, :], AluOp.mult)
            den = pre.tile([128, 1], f32, tag="den")
            nc.vector.reduce_sum(den[0:P, :], sq[0:P, :], axis=mybir.AxisListType.X)
            rden = pre.tile([128, 1], f32, tag="rden")
            nc.vector.reciprocal(rden[0:P, :], den[0:P, :])
            ratio = pre.tile([128, K], f32, tag="ratio")
            nc.vector.tensor_scalar(
                ratio[0:P, :], cnt[0:P, :], rden[0:P, 0:1], None, AluOp.mult
            )
            # r_gm[g, j] = ratio[g, cls[g, j]] = sum_c masks[g,c,j]*ratio[g,c]
            rgt = pre.tile([128, K * NPG], bf16, tag="rgt")
            rgt_v = rgt[0:P, :].rearrange("p (c j) -> p c j", j=NPG)
            nc.vector.tensor_tensor(
                rgt_v,
                masks_v,
                ratio[0:P, :, None].broadcast_to([P, K, NPG]),
                AluOp.mult,
            )
            r_gm = pre.tile([128, NPG], f32, tag="r_gm")
            nc.vector.reduce_sum(
                r_gm[0:P, :, None],
                rgt[0:P, :].rearrange("p (c j) -> p j c", j=NPG),
                axis=mybir.AxisListType.X,
            )

            # graph-major -> node layout via PE transpose. Tile t holds graph
            # pair (2t, 2t+1): A rows at 0:50, B rows at 64:114.
            cls_n = nodep.tile([128, half], f32, tag="cls_n")
            r_n = nodep.tile([128, half], f32, tag="r_n")
            if si < 3 and _rep == 0:
                # first rotation of the bufs=3 pool: zero junk rows 50:64
                # (inside the matmul's 0:114 read range) so the one-hot and
                # its r_n scale never touch non-finite bits; later rotations
                # inherit these zeros (nothing else writes rows 32:64)
                for t in (cls_n, r_n):
                    nc.vector.memset(t[32:64, :], 0.0)
            for src_gm, dst_n in ((cls_f, cls_n), (r_gm, r_n)):
                tp = ps_tr.tile([NPG, 128], f32, tag="tr")
                nc.tensor.transpose(tp[:, 0:P], src_gm[0:P, 0:NPG], ident[0:P, 0:P])
                tp3 = tp[:, 0:P].rearrange("j (t h) -> j t h", h=2)
                nc.scalar.copy(dst_n[0:NPG, :], tp3[:, :, 0])
                nc.scalar.copy(dst_n[64 : 64 + NPG, :], tp3[:, :, 1])

            # scaled one-hot: oh[p, t, s] = (iota16[p, s] == cls_n[p, t]) *
            # r_n[p, t]; seg offset pre-baked into iota16. Rows 114:128 are
            # never read by the matmuls and stay untouched.
            NR = 64 + NPG
            oh = p1s.tile([128, segs], f32r, tag=f"oh{si}")
            oh_v = oh[0:NR, :].rearrange("p (t s) -> p t s", s=2 * K)
            nc.vector.tensor_tensor(
                oh_v,
                iota16[0:NR, None, :].broadcast_to([NR, half, 2 * K]),
                cls_n[0:NR, :, None].broadcast_to([NR, half, 2 * K]),
                AluOp.is_equal,
            )
            nc.vector.tensor_tensor(
                oh_v,
                oh_v,
                r_n[0:NR, :, None].broadcast_to([NR, half, 2 * K]),
                AluOp.mult,
            )
            a_gm = p1s.tile([128, K], f32, tag=f"a_gm{si}")
            p1[si] = (masks, cnt, oh, a_gm)

        def emit_p2(si):
            nonlocal xt_idx
            P, P_real = sbs[si]
            g0 = sb_g0[si]
            half = P // 2
            segs = 2 * K * half
            masks, cnt, oh, a_gm = p1[si]
            a_row = mlp.tile([1, 1024], f32, tag="a_row")
            # pass A: ALL of this superblock's segment matmuls first, so the
            # last reader of an xt buffer finishes as early as possible and
            # releases the next x-load's WAR wait; the MLP chains (pass B,
            # which stall the PE on Act/DVE round trips) come after.
            groups = []
            t0 = 0
            while t0 < half:
                gt = min(32, half - t0)  # tiles in this MLP group
                gcols = 2 * K * gt
                pnx = ps_nx.tile([F, 512], f32, tag="nx")
                groups.append((t0, gt, gcols, pnx))
                for tk in range(t0, t0 + gt):
                    gp = g0 // 2 + tk  # global pair slot (slots are
                    # contiguous across superblocks; real pairs < n_pairs)
                    if gp % XB_PAIRS == 0:
                        # load the next XB_PAIRS pairs (both parities) into
                        # the ring. One buffer feeds 2 MLP groups: few, huge
                        # DMAs amortize the ~1.2us/DMA HWDGE+SEQ overhead.
                        xt_l = xts[(gp // XB_PAIRS) % N_XT]
                        n_real = max(0, min(XB_PAIRS, n_pairs_real - gp))
                        if n_real > 0 and not SKIP_X_DMA:
                            for hi, prt in (
                                (0, slice(0, NPG)),
                                (1, slice(64, 64 + NPG)),
                            ):
                                dst = xt_l[prt, 0 : n_real * F].rearrange(
                                    "p (t f) -> p t f", f=F
                                )
                                src = x4[gp : gp + n_real, hi, :, :].rearrange(
                                    "g n f -> n g f"
                                )
                                nc.sync.dma_start(dst, src)
                        if n_real < XB_PAIRS:
                            # zero pad-pair columns: matmuls never read junk
                            nc.vector.memset(
                                xt_l[
                                    0 : 64 + NPG, max(n_real, 0) * F :
                                ].bitcast(f32),
                                0.0,
                            )
                    xt = xts[(gp // XB_PAIRS) % N_XT]
                    k = gp % XB_PAIRS
                    nc.tensor.matmul(
                        pnx[:, 2 * K * (tk - t0) : 2 * K * (tk - t0 + 1)],
                        xt[0 : 64 + NPG, k * F : (k + 1) * F],
                        oh[0 : 64 + NPG, 2 * K * tk : 2 * K * (tk + 1)],
                        start=True,
                        stop=True,
                    )
                t0 += gt
            # pass B: the MLP chains for both groups
            for t0, gt, gcols, pnx in groups:
                nxs = mlp.tile([F, 512], bf16, tag="nxs")
                nc.scalar.copy(nxs[:, 0:gcols], pnx[:, 0:gcols])
                ph = ps_h.tile([F, 512], f32, tag="h")
                nc.tensor.matmul(
                    ph[:, 0:gcols], w1_sb[:, :], nxs[:, 0:gcols], start=True, stop=True
                )
                z = mlp.tile([F, 512], bf16, tag="z")
                nc.scalar.activation(
                    z[:, 0:gcols], ph[:, 0:gcols], Act.Identity, bias=b1_sb[:, 0:1]
                )
                hT = mlp.tile([F, 512], bf16, tag="hT")
                nc.vector.scalar_tensor_tensor(
                    hT[:, 0:gcols], z[:, 0:gcols], NEG_SLOPE, z[:, 0:gcols],
                    AluOp.mult, AluOp.max,
                )
                pa = ps_a.tile([1, 512], f32, tag="a")
                nc.tensor.matmul(
                    pa[0:1, 0:gcols], w2_sb[:, 0:1], hT[:, 0:gcols], start=True, stop=True
                )
                nc.scalar.copy(a_row[0:1, 2 * K * t0 : 2 * K * t0 + gcols], pa[0:1, 0:gcols])
            # a_row col 16t+8h+c = 8*(2t+h)+c -> graph-major linear order
            a_src = a_row[0:1, 0:segs].rearrange("p (g c) -> p g c", c=K)
            # Pool-queue (SWDGE) DMA: this scatter waits on the whole MLP
            # chain, and a dma_start holds its issuing sequencer until the
            # wait clears — on the SP queue it would freeze the x-load
            # prefetch stream for ~9us per superblock
            nc.gpsimd.dma_start(a_gm[0:P, :], a_src)

        def emit_p3(si):
            P, P_real = sbs[si]
            g0 = sb_g0[si]
            masks, cnt, oh, a_gm = p1[si]
            masks_v = masks[0:P, :].rearrange("p (c j) -> p c j", j=NPG)
            m = post.tile([128, 1], f32, tag="m")
            nc.vector.reduce_max(m[0:P, :], a_gm[0:P, :], axis=mybir.AxisListType.X)
            negm = post.tile([128, 1], f32, tag="negm")
            nc.vector.tensor_scalar(negm[0:P, :], m[0:P, :], -1.0, None, AluOp.mult)
            e = post.tile([128, K], f32, tag="e")
            nc.scalar.activation(e[0:P, :], a_gm[0:P, :], Act.Exp, bias=negm[0:P, 0:1])
            es = post.tile([128, K], f32, tag="es")
            nc.vector.tensor_tensor(es[0:P, :], e[0:P, :], cnt[0:P, :], AluOp.mult)
            s = post.tile([128, 1], f32, tag="s")
            nc.vector.reduce_sum(s[0:P, :], es[0:P, :], axis=mybir.AxisListType.X)
            sp = post.tile([128, 1], f32, tag="sp")
            nc.vector.tensor_scalar(sp[0:P, :], s[0:P, :], 1e-16, None, AluOp.add)
            rs = post.tile([128, 1], f32, tag="rs")
            nc.vector.reciprocal(rs[0:P, :], sp[0:P, :])
            wtab = post.tile([128, K], f32, tag="wtab")
            nc.vector.tensor_scalar(wtab[0:P, :], e[0:P, :], rs[0:P, 0:1], None, AluOp.mult)

            # w_node[g, j] = wtab[g, cls[g, j]] = sum_c masks[g,c,j]*wtab[g,c]
            wnt = post.tile([128, K * NPG], f32, tag="wnt")
            wnt_v = wnt[0:P, :].rearrange("p (c j) -> p c j", j=NPG)
            nc.vector.tensor_tensor(
                wnt_v,
                masks_v,
                wtab[0:P, :, None].broadcast_to([P, K, NPG]),
                AluOp.mult,
            )
            w_n = post.tile([128, NPG], f32, tag="w_n")
            nc.vector.reduce_sum(
                w_n[0:P, :, None],
                wnt[0:P, :].rearrange("p (c j) -> p j c", j=NPG),
                axis=mybir.AxisListType.X,
            )
            # Pool queue for the same reason as the a_gm scatter: the store
            # waits on w_n and must not stall the SP prefetch stream
            nc.gpsimd.dma_start(out_d[g0 : g0 + P_real, :], w_n[0:P_real, :])

        # software-pipelined emission: lookahead keeps every in-order queue
        # fed — phase2(si) runs while phase1(si+3) fills and phase3(si-1)
        # drains, so no engine waits on a 10-superblock serial prologue.
        LOOK = 2
        n_sb = len(sbs)
        for _rep in range(reps):
            for si in range(min(LOOK, n_sb)):
                emit_p1(si)
            for si in range(n_sb):
                # feed the DVE/Act queues with independent pre-pass and
                # drain work BEFORE this superblock's matmul-dependent ops,
                # so those queues never head-of-line block on the PE chain
                if si + LOOK < n_sb:
                    emit_p1(si + LOOK)
                if si >= 1:
                    emit_p3(si - 1)
                emit_p2(si)
            emit_p3(n_sb - 1)

    _split_waits(nc)
    return nc


def make_const_inputs() -> dict[str, np.ndarray]:
    off = np.full((F, 1), 64.0, dtype=np.float32)
    off[0:NPG] = 0.0
    off[64 : 64 + NPG] = float(K)
    return {
        "ident": np.eye(F, dtype=np.float32),
        "iota16": np.arange(2 * K, dtype=np.float32)[None, :] - off,
        "iotak": np.tile(np.arange(K, dtype=np.float32), (F, 1)),
    }


_NC_CACHE: dict[int, object] = {}
TRACE = False          # test harness sets True to collect an NTFF profile
LAST_RESULTS = None    # BassKernelResults of the most recent run


def _get_nc(g_core: int):
    if g_core not in _NC_CACHE:
        _NC_CACHE[g_core] = build_kernel(g_core)
    return _NC_CACHE[g_core]


def make_in_maps(inputs) -> list[dict[str, np.ndarray]]:
    x = np.ascontiguousarray(np.asarray(inputs["x"], dtype=np.float32))
    cls = np.ascontiguousarray(np.asarray(inputs["cls"], dtype=np.int32))
    w1 = np.asarray(inputs["W1"], dtype=np.float32)
    b1 = np.asarray(inputs["b1"], dtype=np.float32).reshape(F, 1)
    w2 = np.asarray(inputs["W2"], dtype=np.float32).reshape(F, 1)
    assert x.shape[0] == N_TOTAL, f"kernel hardcoded for N={N_TOTAL}"

    import ml_dtypes
    wpk = np.ascontiguousarray(
        np.concatenate([w1, b1, w2], axis=1).astype(ml_dtypes.bfloat16)
    )
    in_maps = []
    for core in range(N_CORES):
        lo, hi = core * N_CORE, (core + 1) * N_CORE
        in_maps.append(
            {
                "x": x[lo:hi],
                "cls": cls[lo:hi].reshape(G_CORE, NPG),
                "wpk": wpk,
            }
        )
    return in_maps


def kernel(**inputs) -> np.ndarray:
    nc = _get_nc(G_CORE)
    in_maps = make_in_maps(inputs)
    res = run_bass_kernel_spmd(nc, in_maps, list(range(N_CORES)), trace=TRACE)
    global LAST_RESULTS
    LAST_RESULTS = res
    outs = [res.results[c]["out"].reshape(N_CORE, 1) for c in range(N_CORES)]
    return np.ascontiguousarray(np.concatenate(outs, axis=0))


if __name__ == "__main__":
    ins = {
        "x": np.random.randn(N_TOTAL, F).astype(np.float32),
        "cls": np.random.randint(0, K, size=N_TOTAL).astype(np.int32),
        "batch": np.repeat(np.arange(G_TOTAL, dtype=np.int32), NPG),
        "W1": (np.random.randn(F, F) * 0.05).astype(np.float32),
        "b1": np.zeros(F, dtype=np.float32),
        "W2": (np.random.randn(F, 1) * 0.05).astype(np.float32),
        "b2": np.zeros(1, dtype=np.float32),
        "num_graphs": G_TOTAL,
        "num_clusters": K,
    }
    out = kernel(**ins)
    print(out.shape, out.dtype, out[:5, 0])



# revision 54
# speedup vs baseline: 1.0315x; 1.0315x over previous
"""Cluster-attention GNN kernel for TRN2 (8 NeuronCores, SPMD data-parallel over graphs).

Math (per graph g of exactly 50 nodes, clusters c in 0..7):
  cnt[g,c]   = #nodes with cls==c
  ratio[g,c] = cnt[g,c] / sum_c cnt[g,c]^2
  newx[g,c]  = ratio[g,c] * sum_{nodes in (g,c)} x          (fold ratio per-node into one-hot)
  h = leaky_relu(newx @ W1 + b1, 0.45); a = h @ W2          (only G*K distinct rows!)
  w[g,c] = exp(a-m_g) / (1e-16 + sum_c cnt[g,c] * exp(a-m_g))   (b2 cancels in softmax)
  out[node] = w[g(node), cls(node)]

Device layout: shard = 1250 graphs/core, superblocks of P<=128 graphs.
Node-tile t of a superblock = graph pair (2t, 2t+1): partitions 0:50 and
64:114. One f32r matmul per tile (single-pass bf16x2 weight load, 3x fp32):
lhsT = x_tile[114,128], rhs = scaled one-hot [114,16] -> psum[:, 16t:16t+16]
accumulates new_x^T feature-major. The MLP runs on 512-column groups in bf16.
x loads are 2 DMAs per 64 pairs (3.2MB each) so per-DMA HWDGE/SEQ overhead
(~1.2us serialized) is negligible. Emission is phase-major software-pipelined
(pre-pass / matmul+MLP / softmax+store with lookahead 3): per-engine queues
are in-order, so independent superblocks' work is interleaved to keep every
queue draining at throughput instead of serializing each superblock's long
cross-engine latency chain. Segment reductions are free-dim DVE broadcasts in
graph-major [P, *] layout; graph-major <-> node layouts bridged by PE
transposes. Relative error ~3e-4 (f32r+bf16) vs the 2e-2 gate.
"""

import os
import sys
from contextlib import ExitStack

import numpy as np

import concourse.bass as bass
import concourse.tile as tile
import concourse.tile_sem_assignment as _tsa
from concourse import mybir
from concourse.bass_utils import run_bass_kernel_spmd

# _split_waits() below hoists extra semaphore waits onto NoOps, so the
# default 8 HWDGE completion lanes (finer-grained DMA tracking, better
# overlap) are fine even where an instruction depends on several DMAs.
_ = _tsa  # imported for potential lane tuning; default NUM_HWDGE_SEMS kept

# CoreSim's race detector requires every instruction to carry an update, but
# walrus's NOP encoding shares one semaphore_value field between wait and
# update and rejects a NOP carrying both. Sim runs set this True.
SIM_NOP_UPDATES = False


def _split_waits(nc):
    """Hoist all-but-one sync wait off each instruction onto NoOps before it.

    This walrus build's TPB instruction encodings have a single EVENTS slot,
    so codegen rejects any instruction carrying more than one semaphore wait
    ("Too many sync wait commands"). A NoOp on the same engine stalls that
    sequencer identically. The NoOp's zero-increment dummy update targets the
    split instruction's own update sem (same writer domain, so the race
    detector stays happy) without advancing it.
    """
    # engine -> a semaphore that engine already updates (same writer domain);
    # fall back to a DMA completion-lane sem (multi-writer by design)
    eng_sem: dict = {}
    dma_sem = None
    for blk in nc.m.functions[0].blocks:
        for ins in blk.instructions:
            si = ins.sync_info
            if not si or not si.on_update:
                continue
            u = si.on_update[0]
            if dma_sem is None and u.ant_name.startswith("DMAHW"):
                dma_sem = u
            if ins.engine not in eng_sem and (
                u.ant_name.split("_")[0] == str(ins.engine).split(".")[-1]
            ):
                eng_sem[ins.engine] = u

    k = 0
    for blk in nc.m.functions[0].blocks:
        new = []
        for ins in blk.instructions:
            si = ins.sync_info
            if si and len(si.on_wait) > 1:
                own = (
                    si.on_update[0]
                    if si.on_update
                    else eng_sem.get(ins.engine) or dma_sem
                )
                if own is None:
                    new.append(ins)
                    continue
                for w in si.on_wait[:-1]:
                    nop = mybir.InstNoOp(name=f"split-wait-{k}", ins=[], outs=[])
                    k += 1
                    nop.engine = ins.engine
                    upds = []
                    if SIM_NOP_UPDATES:
                        upds.append(
                            mybir.SyncUpdate(
                                sync_type="semaphore", id=own.id,
                                ant_name=own.ant_name, update_mode="sem-add-imm",
                                update_value=0, update_reg=None,
                            )
                        )
                    nop.sync_info = mybir.SyncInfo(on_wait=[w], on_update=upds)
                    new.append(nop)
                ins.sync_info = mybir.SyncInfo(
                    on_wait=[si.on_wait[-1]], on_update=si.on_update
                )
            new.append(ins)
        blk.instructions = new

F = 128          # feature dim
K = 8            # clusters per graph
NPG = 50         # nodes per graph
NEG_SLOPE = 0.45
N_CORES = 8
SKIP_X_DMA = False   # benchmarking aid: elide x loads (results garbage)

N_TOTAL = 500_000
G_TOTAL = 10_000
G_CORE = G_TOTAL // N_CORES        # 1250 graphs per core
N_CORE = N_TOTAL // N_CORES        # 62500 nodes per core

AluOp = mybir.AluOpType
Act = mybir.ActivationFunctionType
f32 = mybir.dt.float32
f32r = mybir.dt.float32r
bf16 = mybir.dt.bfloat16
i32 = mybir.dt.int32


def superblock_sizes(g_core: int) -> list[tuple[int, int]]:
    """Split g_core graphs into superblocks (P_pad, P_real) with P_pad in {64, 128}.

    P_pad must be 64 or 128 so the graph-major->node-layout PE transposes read
    at base partitions {0, 32, 64} (hardware row-group constraint). Trailing
    graphs are padded with dummy graphs (cls=0, x=0) up to P_pad.
    """
    sizes = []
    g = g_core
    while g > 0:
        if g >= 128:
            sizes.append((128, 128))
            g -= 128
        elif g > 64:
            # one padded 128-superblock beats two half-width ones: each
            # superblock pays a fixed pre/transpose/softmax chain cost
            sizes.append((128, g))
            g = 0
        else:
            sizes.append((64, g))
            g = 0
    return sizes


def build_kernel(g_core: int, reps: int = 1):
    """Build the per-core Bass program for a shard of g_core graphs.

    reps>1 repeats the whole computation in one NEFF (same inputs/outputs,
    idempotent) — used only for benchmarking: the marginal time slope over
    reps cancels the ~600us per-dispatch overhead of the axon PJRT path.
    """
    nc = bass.Bass("TRN2", target_bir_lowering=False, debug=False)

    n_core = g_core * NPG
    x_d = nc.dram_tensor("x", [n_core, F], f32r, kind="ExternalInput").ap()
    cls_d = nc.dram_tensor("cls", [g_core, NPG], i32, kind="ExternalInput").ap()
    # W1 | b1 | W2 packed into one [F, F+2] operand (fewer PJRT operands =
    # less per-dispatch overhead); fixed lookup tables are NEFF-embedded
    # Const tensors, not runtime inputs.
    wpk_d = nc.dram_tensor("wpk", [F, F + 2], bf16, kind="ExternalInput").ap()
    consts = make_const_inputs()
    ident_d = nc.inline_tensor(consts["ident"], name="identc").ap()
    iota16_d = nc.inline_tensor(consts["iota16"], name="iota16c").ap()
    iotak_d = nc.inline_tensor(consts["iotak"], name="iotakc").ap()
    out_d = nc.dram_tensor("out", [g_core, NPG], f32, kind="ExternalOutput").ap()

    sbs = superblock_sizes(g_core)

    with tile.TileContext(nc) as tc, ExitStack() as ctx:
        const = ctx.enter_context(tc.tile_pool(name="const", bufs=1))
        p1s = ctx.enter_context(tc.tile_pool(name="p1s", bufs=1))
        pre = ctx.enter_context(tc.tile_pool(name="pre", bufs=3))
        nodep = ctx.enter_context(tc.tile_pool(name="nodep", bufs=3))
        mlp = ctx.enter_context(tc.tile_pool(name="mlp", bufs=3))
        post = ctx.enter_context(tc.tile_pool(name="post", bufs=3))
        ps_nx = ctx.enter_context(tc.tile_pool(name="ps_nx", bufs=2, space="PSUM"))
        ps_h = ctx.enter_context(tc.tile_pool(name="ps_h", bufs=2, space="PSUM"))
        ps_a = ctx.enter_context(tc.tile_pool(name="ps_a", bufs=2, space="PSUM"))
        ps_tr = ctx.enter_context(tc.tile_pool(name="ps_tr", bufs=2, space="PSUM"))

        # --- constants ---
        wpk_sb = const.tile([F, F + 2], bf16, tag="wpk")
        nc.sync.dma_start(wpk_sb[:, :], wpk_d)
        w1_sb = wpk_sb[:, 0:F]
        b1_sb = wpk_sb[:, F : F + 1]
        w2_sb = wpk_sb[:, F + 1 : F + 2]
        ident = const.tile([F, F], f32, tag="ident")
        nc.sync.dma_start(ident[:, :], ident_d)
        # iota16[p, s] = s - seg_offset(p): rows 0:50 -> s (graph A), rows
        # 64:114 -> s-8 (graph B), junk rows -> s-64 (never equals a cls in
        # 0..7, so the one-hot is 0 there)
        iota16 = const.tile([F, 2 * K], f32, tag="iota16")
        nc.sync.dma_start(iota16[:, :], iota16_d)
        iotak = const.tile([F, K], f32, tag="iotak")  # row = [0..7] everywhere
        nc.sync.dma_start(iotak[:, :], iotak_d)

        # Node-layout x tiles: graph pair (2t, 2t+1) at partitions {0:50,
        # 64:114} (A, B). One DMA per parity loads XB_PAIRS pairs (one MLP
        # group): fewer, bigger DMAs so the ~1.2us serialized per-DMA
        # HWDGE+SEQ overhead amortizes over 800KB instead of 200KB. Junk
        # partitions 50:64 zeroed once (matmul reads 0:114).
        XB_PAIRS = 64
        N_XT = 2
        # PE weight loads are 3x faster for float32r (single-pass bf16x2)
        # than float32, and f32r is byte-identical to f32 — so x and the
        # one-hot run the segment matmuls in f32r with zero conversion ops.
        xts = [
            const.tile([F, XB_PAIRS * F], f32r, tag=f"xt{i}", name=f"xt{i}")
            for i in range(N_XT)
        ]
        for t in xts:
            # junk partitions: one-time zero on the Pool engine so matmul
            # reads of rows 50:64 never see non-finite bits (f32 bitcast:
            # Pool memset has no f32r encoding; zero bits are valid f32r)
            nc.gpsimd.memset(t[32:64, :].bitcast(f32), 0.0)
        xt_idx = 0
        x4 = x_d.rearrange(
            "(gp two n) f -> gp two n f", two=2, n=NPG
        )  # [pairs, 2, 50, 128]

      # Phase-major emission: per-engine queues are in-order, so emitting
      # sb0's whole chain before sb1's serializes the long cross-engine
      # latency chain 10x. Instead: phase 1 (pre-pass) for ALL superblocks,
      # then phase 2 (x loads + matmuls + MLP), then phase 3 (softmax +
      # store). Within a phase, consecutive superblocks are independent, so
      # every queue drains at engine throughput and phases overlap naturally
      # through the queues.
        sb_g0 = []
        g0 = 0
        for P, P_real in sbs:
            sb_g0.append(g0)
            g0 += P_real
        n_pairs_real = (g0 + 1) // 2  # real graph pairs (g_core/2)

        p1 = {}

        def emit_p1(si):
            P, P_real = sbs[si]
            g0 = sb_g0[si]
            half = P // 2
            segs = 2 * K * half  # one-hot columns in this superblock

            cls_i = pre.tile([128, NPG], i32, tag="cls_i")
            nc.sync.dma_start(cls_i[0:P_real, :], cls_d[g0 : g0 + P_real, :])
            cls_f = pre.tile([128, NPG], f32, tag="cls_f")
            if P_real < P:
                nc.vector.memset(cls_f[0:P, :], 0.0)  # pad graphs: all cluster 0
            nc.vector.tensor_copy(cls_f[0:P_real, :], cls_i[0:P_real, :])

            # masks[g, c, j] = (cls[g, j] == c), one broadcast is_equal
            masks = p1s.tile([128, K * NPG], bf16, tag=f"masks{si}")
            masks_v = masks[0:P, :].rearrange("p (c j) -> p c j", j=NPG)
            nc.vector.tensor_tensor(
                masks_v,
                iotak[0:P, :, None].broadcast_to([P, K, NPG]),
                cls_f[0:P, None, :].broadcast_to([P, K, NPG]),
                AluOp.is_equal,
            )
            cnt = p1s.tile([128, K], f32, tag=f"cnt{si}")
            nc.vector.reduce_sum(
                cnt[0:P, :, None], masks_v, axis=mybir.AxisListType.X
            )
            sq = pre.tile([128, K], f32, tag="sq")
            nc.vector.tensor_tensor(sq[0:P, :], cnt[0:P, :], cnt[0:P, :], AluOp.mult)
            den = pre.tile([128, 1], f32, tag="den")
            nc.vector.reduce_sum(den[0:P, :], sq[0:P, :], axis=mybir.AxisListType.X)
            rden = pre.tile([128, 1], f32, tag="rden")
            nc.vector.reciprocal(rden[0:P, :], den[0:P, :])
            ratio = pre.tile([128, K], f32, tag="ratio")
            nc.vector.tensor_scalar(
                ratio[0:P, :], cnt[0:P, :], rden[0:P, 0:1], None, AluOp.mult
            )
            # r_gm[g, j] = ratio[g, cls[g, j]] = sum_c masks[g,c,j]*ratio[g,c]
            rgt = pre.tile([128, K * NPG], bf16, tag="rgt")
            rgt_v = rgt[0:P, :].rearrange("p (c j) -> p c j", j=NPG)
            nc.vector.tensor_tensor(
                rgt_v,
                masks_v,
                ratio[0:P, :, None].broadcast_to([P, K, NPG]),
                AluOp.mult,
            )
            r_gm = pre.tile([128, NPG], f32, tag="r_gm")
            nc.vector.reduce_sum(
                r_gm[0:P, :, None],
                rgt[0:P, :].rearrange("p (c j) -> p j c", j=NPG),
                axis=mybir.AxisListType.X,
            )

            # graph-major -> node layout via PE transpose. Tile t holds graph
            # pair (2t, 2t+1): A rows at 0:50, B rows at 64:114.
            cls_n = nodep.tile([128, half], f32, tag="cls_n")
            r_n = nodep.tile([128, half], f32, tag="r_n")
            if si < 3 and _rep == 0:
                # first rotation of the bufs=3 pool: zero junk rows 50:64
                # (inside the matmul's 0:114 read range) so the one-hot and
                # its r_n scale never touch non-finite bits; later rotations
                # inherit these zeros (nothing else writes rows 32:64)
                for t in (cls_n, r_n):
                    nc.vector.memset(t[32:64, :], 0.0)
            for src_gm, dst_n in ((cls_f, cls_n), (r_gm, r_n)):
                tp = ps_tr.tile([NPG, 128], f32, tag="tr")
                nc.tensor.transpose(tp[:, 0:P], src_gm[0:P, 0:NPG], ident[0:P, 0:P])
                tp3 = tp[:, 0:P].rearrange("j (t h) -> j t h", h=2)
                nc.scalar.copy(dst_n[0:NPG, :], tp3[:, :, 0])
                nc.scalar.copy(dst_n[64 : 64 + NPG, :], tp3[:, :, 1])

            # scaled one-hot: oh[p, t, s] = (iota16[p, s] == cls_n[p, t]) *
            # r_n[p, t]; seg offset pre-baked into iota16. Rows 114:128 are
            # never read by the matmuls and stay untouched.
            NR = 64 + NPG
            oh = p1s.tile([128, segs], f32r, tag=f"oh{si}")
            oh_v = oh[0:NR, :].rearrange("p (t s) -> p t s", s=2 * K)
            nc.vector.tensor_tensor(
                oh_v,
                iota16[0:NR, None, :].broadcast_to([NR, half, 2 * K]),
                cls_n[0:NR, :, None].broadcast_to([NR, half, 2 * K]),
                AluOp.is_equal,
            )
            nc.vector.tensor_tensor(
                oh_v,
                oh_v,
                r_n[0:NR, :, None].broadcast_to([NR, half, 2 * K]),
                AluOp.mult,
            )
            a_gm = p1s.tile([128, K], f32, tag=f"a_gm{si}")
            p1[si] = (masks, cnt, oh, a_gm)

        def emit_p2(si):
            nonlocal xt_idx
            P, P_real = sbs[si]
            g0 = sb_g0[si]
            half = P // 2
            segs = 2 * K * half
            masks, cnt, oh, a_gm = p1[si]
            a_row = mlp.tile([1, 1024], f32, tag="a_row")
            # pass A: ALL of this superblock's segment matmuls first, so the
            # last reader of an xt buffer finishes as early as possible and
            # releases the next x-load's WAR wait; the MLP chains (pass B,
            # which stall the PE on Act/DVE round trips) come after.
            groups = []
            t0 = 0
            while t0 < half:
                gt = min(32, half - t0)  # tiles in this MLP group
                gcols = 2 * K * gt
                pnx = ps_nx.tile([F, 512], f32, tag="nx")
                groups.append((t0, gt, gcols, pnx))
                for tk in range(t0, t0 + gt):
                    gp = g0 // 2 + tk  # global pair slot (slots are
                    # contiguous across superblocks; real pairs < n_pairs)
                    if gp % XB_PAIRS == 0:
                        # load the next XB_PAIRS pairs (both parities) into
                        # the ring. One buffer feeds 2 MLP groups: few, huge
                        # DMAs amortize the ~1.2us/DMA HWDGE+SEQ overhead.
                        xt_l = xts[(gp // XB_PAIRS) % N_XT]
                        n_real = max(0, min(XB_PAIRS, n_pairs_real - gp))
                        if n_real > 0 and not SKIP_X_DMA:
                            for hi, prt in (
                                (0, slice(0, NPG)),
                                (1, slice(64, 64 + NPG)),
                            ):
                                dst = xt_l[prt, 0 : n_real * F].rearrange(
                                    "p (t f) -> p t f", f=F
                                )
                                src = x4[gp : gp + n_real, hi, :, :].rearrange(
                                    "g n f -> n g f"
                                )
                                nc.sync.dma_start(dst, src)
                        if n_real < XB_PAIRS:
                            # zero pad-pair columns: matmuls never read junk
                            nc.vector.memset(
                                xt_l[
                                    0 : 64 + NPG, max(n_real, 0) * F :
                                ].bitcast(f32),
                                0.0,
                            )
                    xt = xts[(gp // XB_PAIRS) % N_XT]
                    k = gp % XB_PAIRS
                    nc.tensor.matmul(
                        pnx[:, 2 * K * (tk - t0) : 2 * K * (tk - t0 + 1)],
                        xt[0 : 64 + NPG, k * F : (k + 1) * F],
                        oh[0 : 64 + NPG, 2 * K * tk : 2 * K * (tk + 1)],
                        start=True,
                        stop=True,
                    )
                t0 += gt
            # pass B: the MLP chains for both groups
            for t0, gt, gcols, pnx in groups:
                nxs = mlp.tile([F, 512], bf16, tag="nxs")
                nc.scalar.copy(nxs[:, 0:gcols], pnx[:, 0:gcols])
                ph = ps_h.tile([F, 512], f32, tag="h")
                nc.tensor.matmul(
                    ph[:, 0:gcols], w1_sb[:, :], nxs[:, 0:gcols], start=True, stop=True
                )
                z = mlp.tile([F, 512], bf16, tag="z")
                nc.scalar.activation(
                    z[:, 0:gcols], ph[:, 0:gcols], Act.Identity, bias=b1_sb[:, 0:1]
                )
                hT = mlp.tile([F, 512], bf16, tag="hT")
                nc.vector.scalar_tensor_tensor(
                    hT[:, 0:gcols], z[:, 0:gcols], NEG_SLOPE, z[:, 0:gcols],
                    AluOp.mult, AluOp.max,
                )
                pa = ps_a.tile([1, 512], f32, tag="a")
                nc.tensor.matmul(
                    pa[0:1, 0:gcols], w2_sb[:, 0:1], hT[:, 0:gcols], start=True, stop=True
                )
                nc.scalar.copy(a_row[0:1, 2 * K * t0 : 2 * K * t0 + gcols], pa[0:1, 0:gcols])
            # a_row col 16t+8h+c = 8*(2t+h)+c -> graph-major linear order
            a_src = a_row[0:1, 0:segs].rearrange("p (g c) -> p g c", c=K)
            # Pool-queue (SWDGE) DMA: this scatter waits on the whole MLP
            # chain, and a dma_start holds its issuing sequencer until the
            # wait clears — on the SP queue it would freeze the x-load
            # prefetch stream for ~9us per superblock
            nc.gpsimd.dma_start(a_gm[0:P, :], a_src)

        def emit_p3(si):
            P, P_real = sbs[si]
            g0 = sb_g0[si]
            masks, cnt, oh, a_gm = p1[si]
            masks_v = masks[0:P, :].rearrange("p (c j) -> p c j", j=NPG)
            m = post.tile([128, 1], f32, tag="m")
            nc.vector.reduce_max(m[0:P, :], a_gm[0:P, :], axis=mybir.AxisListType.X)
            negm = post.tile([128, 1], f32, tag="negm")
            nc.vector.tensor_scalar(negm[0:P, :], m[0:P, :], -1.0, None, AluOp.mult)
            e = post.tile([128, K], f32, tag="e")
            nc.scalar.activation(e[0:P, :], a_gm[0:P, :], Act.Exp, bias=negm[0:P, 0:1])
            es = post.tile([128, K], f32, tag="es")
            nc.vector.tensor_tensor(es[0:P, :], e[0:P, :], cnt[0:P, :], AluOp.mult)
            s = post.tile([128, 1], f32, tag="s")
            nc.vector.reduce_sum(s[0:P, :], es[0:P, :], axis=mybir.AxisListType.X)
            sp = post.tile([128, 1], f32, tag="sp")
            nc.vector.tensor_scalar(sp[0:P, :], s[0:P, :], 1e-16, None, AluOp.add)
            rs = post.tile([128, 1], f32, tag="rs")
            nc.vector.reciprocal(rs[0:P, :], sp[0:P, :])
            wtab = post.tile([128, K], f32, tag="wtab")
            nc.vector.tensor_scalar(wtab[0:P, :], e[0:P, :], rs[0:P, 0:1], None, AluOp.mult)

            # w_node[g, j] = wtab[g, cls[g, j]] = sum_c masks[g,c,j]*wtab[g,c]
            wnt = post.tile([128, K * NPG], f32, tag="wnt")
            wnt_v = wnt[0:P, :].rearrange("p (c j) -> p c j", j=NPG)
            nc.vector.tensor_tensor(
                wnt_v,
                masks_v,
                wtab[0:P, :, None].broadcast_to([P, K, NPG]),
                AluOp.mult,
            )
            w_n = post.tile([128, NPG], f32, tag="w_n")
            nc.vector.reduce_sum(
                w_n[0:P, :, None],
                wnt[0:P, :].rearrange("p (c j) -> p j c", j=NPG),
                axis=mybir.AxisListType.X,
            )
            # Pool queue for the same reason as the a_gm scatter: the store
            # waits on w_n and must not stall the SP prefetch stream
            nc.gpsimd.dma_start(out_d[g0 : g0 + P_real, :], w_n[0:P_real, :])

        # software-pipelined emission: lookahead keeps every in-order queue
        # fed — phase2(si) runs while phase1(si+3) fills and phase3(si-1)
        # drains, so no engine waits on a 10-superblock serial prologue.
        LOOK = 2
        n_sb = len(sbs)
        for _rep in range(reps):
            for si in range(min(LOOK, n_sb)):
                emit_p1(si)
            for si in range(n_sb):
                # feed the DVE/Act queues with independent pre-pass and
                # drain work BEFORE this superblock's matmul-dependent ops,
                # so those queues never head-of-line block on the PE chain
                if si + LOOK < n_sb:
                    emit_p1(si + LOOK)
                if si >= 1:
                    emit_p3(si - 1)
                emit_p2(si)
            emit_p3(n_sb - 1)

    _split_waits(nc)
    return nc


def make_const_inputs() -> dict[str, np.ndarray]:
    off = np.full((F, 1), 64.0, dtype=np.float32)
    off[0:NPG] = 0.0
    off[64 : 64 + NPG] = float(K)
    return {
        "ident": np.eye(F, dtype=np.float32),
        "iota16": np.arange(2 * K, dtype=np.float32)[None, :] - off,
        "iotak": np.tile(np.arange(K, dtype=np.float32), (F, 1)),
    }


_NC_CACHE: dict[int, object] = {}
TRACE = False          # test harness sets True to collect an NTFF profile
LAST_RESULTS = None    # BassKernelResults of the most recent run


def _get_nc(g_core: int):
    if g_core not in _NC_CACHE:
        _NC_CACHE[g_core] = build_kernel(g_core)
    return _NC_CACHE[g_core]


def make_in_maps(inputs) -> list[dict[str, np.ndarray]]:
    x = np.ascontiguousarray(np.asarray(inputs["x"], dtype=np.float32))
    cls = np.ascontiguousarray(np.asarray(inputs["cls"], dtype=np.int32))
    w1 = np.asarray(inputs["W1"], dtype=np.float32)
    b1 = np.asarray(inputs["b1"], dtype=np.float32).reshape(F, 1)
    w2 = np.asarray(inputs["W2"], dtype=np.float32).reshape(F, 1)
    assert x.shape[0] == N_TOTAL, f"kernel hardcoded for N={N_TOTAL}"

    import ml_dtypes
    wpk = np.ascontiguousarray(
        np.concatenate([w1, b1, w2], axis=1).astype(ml_dtypes.bfloat16)
    )
    in_maps = []
    for core in range(N_CORES):
        lo, hi = core * N_CORE, (core + 1) * N_CORE
        in_maps.append(
            {
                "x": x[lo:hi],
                "cls": cls[lo:hi].reshape(G_CORE, NPG),
                "wpk": wpk,
            }
        )
    return in_maps


def kernel(**inputs) -> np.ndarray:
    nc = _get_nc(G_CORE)
    in_maps = make_in_maps(inputs)
    res = run_bass_kernel_spmd(nc, in_maps, list(range(N_CORES)), trace=TRACE)
    global LAST_RESULTS
    LAST_RESULTS = res
    outs = [res.results[c]["out"].reshape(N_CORE, 1) for c in range(N_CORES)]
    return np.ascontiguousarray(np.concatenate(outs, axis=0))


if __name__ == "__main__":
    ins = {
        "x": np.random.randn(N_TOTAL, F).astype(np.float32),
        "cls": np.random.randint(0, K, size=N_TOTAL).astype(np.int32),
        "batch": np.repeat(np.arange(G_TOTAL, dtype=np.int32), NPG),
        "W1": (np.random.randn(F, F) * 0.05).astype(np.float32),
        "b1": np.zeros(F, dtype=np.float32),
        "W2": (np.random.randn(F, 1) * 0.05).astype(np.float32),
        "b2": np.zeros(1, dtype=np.float32),
        "num_graphs": G_TOTAL,
        "num_clusters": K,
    }
    out = kernel(**ins)
    print(out.shape, out.dtype, out[:5, 0])



# revision 58
# speedup vs baseline: 1.1970x; 1.1605x over previous
"""Cluster-attention GNN kernel for TRN2 (8 NeuronCores, SPMD data-parallel over graphs).

Math (per graph g of exactly 50 nodes, clusters c in 0..7):
  cnt[g,c]   = #nodes with cls==c
  ratio[g,c] = cnt[g,c] / sum_c cnt[g,c]^2
  newx[g,c]  = ratio[g,c] * sum_{nodes in (g,c)} x          (fold ratio per-node into one-hot)
  h = leaky_relu(newx @ W1 + b1, 0.45); a = h @ W2          (only G*K distinct rows!)
  w[g,c] = exp(a-m_g) / (1e-16 + sum_c cnt[g,c] * exp(a-m_g))   (b2 cancels in softmax)
  out[node] = w[g(node), cls(node)]

Device layout: shard = 1250 graphs/core, superblocks of P<=128 graphs.
Node-tile t of a superblock = graph pair (2t, 2t+1): partitions 0:50 and
64:114. One f32r matmul per tile (single-pass bf16x2 weight load, 3x fp32):
lhsT = x_tile[114,128], rhs = scaled one-hot [114,16] -> psum[:, 16t:16t+16]
accumulates new_x^T feature-major. The MLP runs on 512-column groups in bf16.
x loads are 2 DMAs per 64 pairs (3.2MB each) so per-DMA HWDGE/SEQ overhead
(~1.2us serialized) is negligible. Emission is phase-major software-pipelined
(pre-pass / matmul+MLP / softmax+store with lookahead 3): per-engine queues
are in-order, so independent superblocks' work is interleaved to keep every
queue draining at throughput instead of serializing each superblock's long
cross-engine latency chain. Segment reductions are free-dim DVE broadcasts in
graph-major [P, *] layout; graph-major <-> node layouts bridged by PE
transposes. Relative error ~3e-4 (f32r+bf16) vs the 2e-2 gate.
"""

import os
import sys
from contextlib import ExitStack

import numpy as np

import concourse.bass as bass
import concourse.tile as tile
import concourse.tile_sem_assignment as _tsa
from concourse import mybir
from concourse.bass_utils import run_bass_kernel_spmd

# _split_waits() below hoists extra semaphore waits onto NoOps, so the
# default 8 HWDGE completion lanes (finer-grained DMA tracking, better
# overlap) are fine even where an instruction depends on several DMAs.
_ = _tsa  # imported for potential lane tuning; default NUM_HWDGE_SEMS kept

# CoreSim's race detector requires every instruction to carry an update, but
# walrus's NOP encoding shares one semaphore_value field between wait and
# update and rejects a NOP carrying both. Sim runs set this True.
SIM_NOP_UPDATES = False


def _split_waits(nc):
    """Hoist all-but-one sync wait off each instruction onto NoOps before it.

    This walrus build's TPB instruction encodings have a single EVENTS slot,
    so codegen rejects any instruction carrying more than one semaphore wait
    ("Too many sync wait commands"). A NoOp on the same engine stalls that
    sequencer identically. The NoOp's zero-increment dummy update targets the
    split instruction's own update sem (same writer domain, so the race
    detector stays happy) without advancing it.
    """
    # engine -> a semaphore that engine already updates (same writer domain);
    # fall back to a DMA completion-lane sem (multi-writer by design)
    eng_sem: dict = {}
    dma_sem = None
    for blk in nc.m.functions[0].blocks:
        for ins in blk.instructions:
            si = ins.sync_info
            if not si or not si.on_update:
                continue
            u = si.on_update[0]
            if dma_sem is None and u.ant_name.startswith("DMAHW"):
                dma_sem = u
            if ins.engine not in eng_sem and (
                u.ant_name.split("_")[0] == str(ins.engine).split(".")[-1]
            ):
                eng_sem[ins.engine] = u

    k = 0
    for blk in nc.m.functions[0].blocks:
        new = []
        for ins in blk.instructions:
            si = ins.sync_info
            if si and len(si.on_wait) > 1:
                own = (
                    si.on_update[0]
                    if si.on_update
                    else eng_sem.get(ins.engine) or dma_sem
                )
                if own is None:
                    new.append(ins)
                    continue
                for w in si.on_wait[:-1]:
                    nop = mybir.InstNoOp(name=f"split-wait-{k}", ins=[], outs=[])
                    k += 1
                    nop.engine = ins.engine
                    upds = []
                    if SIM_NOP_UPDATES:
                        upds.append(
                            mybir.SyncUpdate(
                                sync_type="semaphore", id=own.id,
                                ant_name=own.ant_name, update_mode="sem-add-imm",
                                update_value=0, update_reg=None,
                            )
                        )
                    nop.sync_info = mybir.SyncInfo(on_wait=[w], on_update=upds)
                    new.append(nop)
                ins.sync_info = mybir.SyncInfo(
                    on_wait=[si.on_wait[-1]], on_update=si.on_update
                )
            new.append(ins)
        blk.instructions = new

F = 128          # feature dim
K = 8            # clusters per graph
NPG = 50         # nodes per graph
NEG_SLOPE = 0.45
N_CORES = 8
SKIP_X_DMA = False   # benchmarking aid: elide x loads (results garbage)

N_TOTAL = 500_000
G_TOTAL = 10_000
G_CORE = G_TOTAL // N_CORES        # 1250 graphs per core
N_CORE = N_TOTAL // N_CORES        # 62500 nodes per core

AluOp = mybir.AluOpType
Act = mybir.ActivationFunctionType
f32 = mybir.dt.float32
f32r = mybir.dt.float32r
bf16 = mybir.dt.bfloat16
i32 = mybir.dt.int32


def superblock_sizes(g_core: int) -> list[tuple[int, int]]:
    """Split g_core graphs into superblocks (P_pad, P_real) with P_pad in {64, 128}.

    P_pad must be 64 or 128 so the graph-major->node-layout PE transposes read
    at base partitions {0, 32, 64} (hardware row-group constraint). Trailing
    graphs are padded with dummy graphs (cls=0, x=0) up to P_pad.
    """
    sizes = []
    g = g_core
    while g > 0:
        if g >= 128:
            sizes.append((128, 128))
            g -= 128
        elif g > 64:
            # one padded 128-superblock beats two half-width ones: each
            # superblock pays a fixed pre/transpose/softmax chain cost
            sizes.append((128, g))
            g = 0
        else:
            sizes.append((64, g))
            g = 0
    return sizes


def build_kernel(g_core: int, reps: int = 1):
    """Build the per-core Bass program for a shard of g_core graphs.

    reps>1 repeats the whole computation in one NEFF (same inputs/outputs,
    idempotent) — used only for benchmarking: the marginal time slope over
    reps cancels the ~600us per-dispatch overhead of the axon PJRT path.
    """
    nc = bass.Bass("TRN2", target_bir_lowering=False, debug=False)

    n_core = g_core * NPG
    x_d = nc.dram_tensor("x", [n_core, F], f32r, kind="ExternalInput").ap()
    cls_d = nc.dram_tensor("cls", [g_core, NPG], i32, kind="ExternalInput").ap()
    # W1 | b1 | W2 packed into one [F, F+2] operand (fewer PJRT operands =
    # less per-dispatch overhead); fixed lookup tables are NEFF-embedded
    # Const tensors, not runtime inputs.
    wpk_d = nc.dram_tensor("wpk", [F, F + 2], bf16, kind="ExternalInput").ap()
    consts = make_const_inputs()
    ident_d = nc.inline_tensor(consts["ident"], name="identc").ap()
    iota16_d = nc.inline_tensor(consts["iota16"], name="iota16c").ap()
    iotak_d = nc.inline_tensor(consts["iotak"], name="iotakc").ap()
    out_d = nc.dram_tensor("out", [g_core, NPG], f32, kind="ExternalOutput").ap()

    sbs = superblock_sizes(g_core)

    with tile.TileContext(nc) as tc, ExitStack() as ctx:
        const = ctx.enter_context(tc.tile_pool(name="const", bufs=1))
        p1s = ctx.enter_context(tc.tile_pool(name="p1s", bufs=1))
        pre = ctx.enter_context(tc.tile_pool(name="pre", bufs=3))
        nodep = ctx.enter_context(tc.tile_pool(name="nodep", bufs=3))
        mlp = ctx.enter_context(tc.tile_pool(name="mlp", bufs=3))
        post = ctx.enter_context(tc.tile_pool(name="post", bufs=3))
        ps_nx = ctx.enter_context(tc.tile_pool(name="ps_nx", bufs=2, space="PSUM"))
        ps_h = ctx.enter_context(tc.tile_pool(name="ps_h", bufs=2, space="PSUM"))
        ps_a = ctx.enter_context(tc.tile_pool(name="ps_a", bufs=2, space="PSUM"))
        ps_tr = ctx.enter_context(tc.tile_pool(name="ps_tr", bufs=2, space="PSUM"))

        # --- constants ---
        wpk_sb = const.tile([F, F + 2], bf16, tag="wpk")
        nc.sync.dma_start(wpk_sb[:, :], wpk_d)
        w1_sb = wpk_sb[:, 0:F]
        b1_sb = wpk_sb[:, F : F + 1]
        w2_sb = wpk_sb[:, F + 1 : F + 2]
        ident = const.tile([F, F], f32, tag="ident")
        nc.sync.dma_start(ident[:, :], ident_d)
        # iota16[p, s] = s - seg_offset(p): rows 0:50 -> s (graph A), rows
        # 64:114 -> s-8 (graph B), junk rows -> s-64 (never equals a cls in
        # 0..7, so the one-hot is 0 there)
        iota16 = const.tile([F, 2 * K], f32, tag="iota16")
        nc.sync.dma_start(iota16[:, :], iota16_d)
        iotak = const.tile([F, K], f32, tag="iotak")  # row = [0..7] everywhere
        nc.sync.dma_start(iotak[:, :], iotak_d)

        # Node-layout x tiles: graph pair (2t, 2t+1) at partitions {0:50,
        # 64:114} (A, B). One DMA per parity loads XB_PAIRS pairs (one MLP
        # group): fewer, bigger DMAs so the ~1.2us serialized per-DMA
        # HWDGE+SEQ overhead amortizes over 800KB instead of 200KB. Junk
        # partitions 50:64 zeroed once (matmul reads 0:114).
        XB_PAIRS = 64
        N_XT = 2
        # PE weight loads are 3x faster for float32r (single-pass bf16x2)
        # than float32, and f32r is byte-identical to f32 — so x and the
        # one-hot run the segment matmuls in f32r with zero conversion ops.
        xts = [
            const.tile([F, XB_PAIRS * F], f32r, tag=f"xt{i}", name=f"xt{i}")
            for i in range(N_XT)
        ]
        for t in xts:
            # junk partitions: one-time zero on the Pool engine so matmul
            # reads of rows 50:64 never see non-finite bits (f32 bitcast:
            # Pool memset has no f32r encoding; zero bits are valid f32r)
            nc.gpsimd.memset(t[32:64, :].bitcast(f32), 0.0)
        xt_idx = 0
        x4 = x_d.rearrange(
            "(gp two n) f -> gp two n f", two=2, n=NPG
        )  # [pairs, 2, 50, 128]

      # Phase-major emission: per-engine queues are in-order, so emitting
      # sb0's whole chain before sb1's serializes the long cross-engine
      # latency chain 10x. Instead: phase 1 (pre-pass) for ALL superblocks,
      # then phase 2 (x loads + matmuls + MLP), then phase 3 (softmax +
      # store). Within a phase, consecutive superblocks are independent, so
      # every queue drains at engine throughput and phases overlap naturally
      # through the queues.
        sb_g0 = []
        g0 = 0
        for P, P_real in sbs:
            sb_g0.append(g0)
            g0 += P_real
        n_pairs_real = (g0 + 1) // 2  # real graph pairs (g_core/2)

        p1 = {}

        def emit_p1(si):
            P, P_real = sbs[si]
            g0 = sb_g0[si]
            half = P // 2
            segs = 2 * K * half  # one-hot columns in this superblock

            cls_i = pre.tile([128, NPG], i32, tag="cls_i")
            nc.sync.dma_start(cls_i[0:P_real, :], cls_d[g0 : g0 + P_real, :])
            cls_f = pre.tile([128, NPG], f32, tag="cls_f")
            if P_real < P:
                nc.vector.memset(cls_f[0:P, :], 0.0)  # pad graphs: all cluster 0
            nc.vector.tensor_copy(cls_f[0:P_real, :], cls_i[0:P_real, :])

            # masks[g, c, j] = (cls[g, j] == c), one broadcast is_equal
            masks = p1s.tile([128, K * NPG], bf16, tag=f"masks{si}")
            masks_v = masks[0:P, :].rearrange("p (c j) -> p c j", j=NPG)
            nc.vector.tensor_tensor(
                masks_v,
                iotak[0:P, :, None].broadcast_to([P, K, NPG]),
                cls_f[0:P, None, :].broadcast_to([P, K, NPG]),
                AluOp.is_equal,
            )
            cnt = p1s.tile([128, K], f32, tag=f"cnt{si}")
            nc.vector.reduce_sum(
                cnt[0:P, :, None], masks_v, axis=mybir.AxisListType.X
            )
            sq = pre.tile([128, K], f32, tag="sq")
            nc.vector.tensor_tensor(sq[0:P, :], cnt[0:P, :], cnt[0:P, :], AluOp.mult)
            den = pre.tile([128, 1], f32, tag="den")
            nc.vector.reduce_sum(den[0:P, :], sq[0:P, :], axis=mybir.AxisListType.X)
            rden = pre.tile([128, 1], f32, tag="rden")
            nc.vector.reciprocal(rden[0:P, :], den[0:P, :])
            ratio = pre.tile([128, K], f32, tag="ratio")
            nc.vector.tensor_scalar(
                ratio[0:P, :], cnt[0:P, :], rden[0:P, 0:1], None, AluOp.mult
            )
            # r_gm[g, j] = ratio[g, cls[g, j]] = sum_c masks[g,c,j]*ratio[g,c]
            rgt = pre.tile([128, K * NPG], bf16, tag="rgt")
            rgt_v = rgt[0:P, :].rearrange("p (c j) -> p c j", j=NPG)
            nc.vector.tensor_tensor(
                rgt_v,
                masks_v,
                ratio[0:P, :, None].broadcast_to([P, K, NPG]),
                AluOp.mult,
            )
            r_gm = pre.tile([128, NPG], f32, tag="r_gm")
            nc.vector.reduce_sum(
                r_gm[0:P, :, None],
                rgt[0:P, :].rearrange("p (c j) -> p j c", j=NPG),
                axis=mybir.AxisListType.X,
            )

            # graph-major -> node layout via PE transpose. Tile t holds graph
            # pair (2t, 2t+1): A rows at 0:50, B rows at 64:114.
            cls_n = nodep.tile([128, half], f32, tag="cls_n")
            r_n = nodep.tile([128, half], f32, tag="r_n")
            if si < 3 and _rep == 0:
                # first rotation of the bufs=3 pool: zero junk rows 50:64
                # (inside the matmul's 0:114 read range) so the one-hot and
                # its r_n scale never touch non-finite bits; later rotations
                # inherit these zeros (nothing else writes rows 32:64)
                for t in (cls_n, r_n):
                    nc.vector.memset(t[32:64, :], 0.0)
            for src_gm, dst_n in ((cls_f, cls_n), (r_gm, r_n)):
                tp = ps_tr.tile([NPG, 128], f32, tag="tr")
                nc.tensor.transpose(tp[:, 0:P], src_gm[0:P, 0:NPG], ident[0:P, 0:P])
                tp3 = tp[:, 0:P].rearrange("j (t h) -> j t h", h=2)
                nc.scalar.copy(dst_n[0:NPG, :], tp3[:, :, 0])
                nc.scalar.copy(dst_n[64 : 64 + NPG, :], tp3[:, :, 1])

            # scaled one-hot: oh[p, t, s] = (iota16[p, s] == cls_n[p, t]) *
            # r_n[p, t]; seg offset pre-baked into iota16. Rows 114:128 are
            # never read by the matmuls and stay untouched.
            NR = 64 + NPG
            oh = p1s.tile([128, segs], f32r, tag=f"oh{si}")
            oh_v = oh[0:NR, :].rearrange("p (t s) -> p t s", s=2 * K)
            nc.vector.tensor_tensor(
                oh_v,
                iota16[0:NR, None, :].broadcast_to([NR, half, 2 * K]),
                cls_n[0:NR, :, None].broadcast_to([NR, half, 2 * K]),
                AluOp.is_equal,
            )
            nc.vector.tensor_tensor(
                oh_v,
                oh_v,
                r_n[0:NR, :, None].broadcast_to([NR, half, 2 * K]),
                AluOp.mult,
            )
            a_gm = p1s.tile([128, K], f32, tag=f"a_gm{si}")
            p1[si] = (masks, cnt, oh, a_gm)

        def emit_p2(si):
            nonlocal xt_idx
            P, P_real = sbs[si]
            g0 = sb_g0[si]
            half = P // 2
            segs = 2 * K * half
            masks, cnt, oh, a_gm = p1[si]
            a_row = mlp.tile([1, 1024], f32, tag="a_row")
            # pass A: ALL of this superblock's segment matmuls first, so the
            # last reader of an xt buffer finishes as early as possible and
            # releases the next x-load's WAR wait; the MLP chains (pass B,
            # which stall the PE on Act/DVE round trips) come after.
            groups = []
            t0 = 0
            while t0 < half:
                gt = min(32, half - t0)  # tiles in this MLP group
                gcols = 2 * K * gt
                pnx = ps_nx.tile([F, 512], f32, tag="nx")
                groups.append((t0, gt, gcols, pnx))
                for tk in range(t0, t0 + gt):
                    gp = g0 // 2 + tk  # global pair slot (slots are
                    # contiguous across superblocks; real pairs < n_pairs)
                    if gp % XB_PAIRS == 0:
                        # load the next XB_PAIRS pairs (both parities) into
                        # the ring. One buffer feeds 2 MLP groups: few, huge
                        # DMAs amortize the ~1.2us/DMA HWDGE+SEQ overhead.
                        xt_l = xts[(gp // XB_PAIRS) % N_XT]
                        n_real = max(0, min(XB_PAIRS, n_pairs_real - gp))
                        if n_real > 0 and not SKIP_X_DMA:
                            for hi, prt in (
                                (0, slice(0, NPG)),
                                (1, slice(64, 64 + NPG)),
                            ):
                                dst = xt_l[prt, 0 : n_real * F].rearrange(
                                    "p (t f) -> p t f", f=F
                                )
                                src = x4[gp : gp + n_real, hi, :, :].rearrange(
                                    "g n f -> n g f"
                                )
                                nc.sync.dma_start(dst, src)
                        if n_real < XB_PAIRS:
                            # zero pad-pair columns: matmuls never read junk
                            nc.vector.memset(
                                xt_l[
                                    0 : 64 + NPG, max(n_real, 0) * F :
                                ].bitcast(f32),
                                0.0,
                            )
                    xt = xts[(gp // XB_PAIRS) % N_XT]
                    k = gp % XB_PAIRS
                    nc.tensor.matmul(
                        pnx[:, 2 * K * (tk - t0) : 2 * K * (tk - t0 + 1)],
                        xt[0 : 64 + NPG, k * F : (k + 1) * F],
                        oh[0 : 64 + NPG, 2 * K * tk : 2 * K * (tk + 1)],
                        start=True,
                        stop=True,
                    )
                t0 += gt
            # pass B: the MLP chains for both groups
            for t0, gt, gcols, pnx in groups:
                nxs = mlp.tile([F, 512], bf16, tag="nxs")
                nc.scalar.copy(nxs[:, 0:gcols], pnx[:, 0:gcols])
                ph = ps_h.tile([F, 512], f32, tag="h")
                nc.tensor.matmul(
                    ph[:, 0:gcols], w1_sb[:, :], nxs[:, 0:gcols], start=True, stop=True
                )
                z = mlp.tile([F, 512], bf16, tag="z")
                nc.scalar.activation(
                    z[:, 0:gcols], ph[:, 0:gcols], Act.Identity, bias=b1_sb[:, 0:1]
                )
                hT = mlp.tile([F, 512], bf16, tag="hT")
                nc.vector.scalar_tensor_tensor(
                    hT[:, 0:gcols], z[:, 0:gcols], NEG_SLOPE, z[:, 0:gcols],
                    AluOp.mult, AluOp.max,
                )
                pa = ps_a.tile([1, 512], f32, tag="a")
                nc.tensor.matmul(
                    pa[0:1, 0:gcols], w2_sb[:, 0:1], hT[:, 0:gcols], start=True, stop=True
                )
                nc.scalar.copy(a_row[0:1, 2 * K * t0 : 2 * K * t0 + gcols], pa[0:1, 0:gcols])
            # a_row col 16t+8h+c = 8*(2t+h)+c -> graph-major linear order
            a_src = a_row[0:1, 0:segs].rearrange("p (g c) -> p g c", c=K)
            # Pool-queue (SWDGE) DMA: this scatter waits on the whole MLP
            # chain, and a dma_start holds its issuing sequencer until the
            # wait clears — on the SP queue it would freeze the x-load
            # prefetch stream for ~9us per superblock
            nc.gpsimd.dma_start(a_gm[0:P, :], a_src)

        def emit_p3(si):
            P, P_real = sbs[si]
            g0 = sb_g0[si]
            masks, cnt, oh, a_gm = p1[si]
            masks_v = masks[0:P, :].rearrange("p (c j) -> p c j", j=NPG)
            m = post.tile([128, 1], f32, tag="m")
            nc.vector.reduce_max(m[0:P, :], a_gm[0:P, :], axis=mybir.AxisListType.X)
            negm = post.tile([128, 1], f32, tag="negm")
            nc.vector.tensor_scalar(negm[0:P, :], m[0:P, :], -1.0, None, AluOp.mult)
            e = post.tile([128, K], f32, tag="e")
            nc.scalar.activation(e[0:P, :], a_gm[0:P, :], Act.Exp, bias=negm[0:P, 0:1])
            es = post.tile([128, K], f32, tag="es")
            nc.vector.tensor_tensor(es[0:P, :], e[0:P, :], cnt[0:P, :], AluOp.mult)
            s = post.tile([128, 1], f32, tag="s")
            nc.vector.reduce_sum(s[0:P, :], es[0:P, :], axis=mybir.AxisListType.X)
            sp = post.tile([128, 1], f32, tag="sp")
            nc.vector.tensor_scalar(sp[0:P, :], s[0:P, :], 1e-16, None, AluOp.add)
            rs = post.tile([128, 1], f32, tag="rs")
            nc.vector.reciprocal(rs[0:P, :], sp[0:P, :])
            wtab = post.tile([128, K], f32, tag="wtab")
            nc.vector.tensor_scalar(wtab[0:P, :], e[0:P, :], rs[0:P, 0:1], None, AluOp.mult)

            # w_node[g, j] = wtab[g, cls[g, j]] = sum_c masks[g,c,j]*wtab[g,c]
            wnt = post.tile([128, K * NPG], f32, tag="wnt")
            wnt_v = wnt[0:P, :].rearrange("p (c j) -> p c j", j=NPG)
            nc.vector.tensor_tensor(
                wnt_v,
                masks_v,
                wtab[0:P, :, None].broadcast_to([P, K, NPG]),
                AluOp.mult,
            )
            w_n = post.tile([128, NPG], f32, tag="w_n")
            nc.vector.reduce_sum(
                w_n[0:P, :, None],
                wnt[0:P, :].rearrange("p (c j) -> p j c", j=NPG),
                axis=mybir.AxisListType.X,
            )
            # Pool queue for the same reason as the a_gm scatter: the store
            # waits on w_n and must not stall the SP prefetch stream
            nc.gpsimd.dma_start(out_d[g0 : g0 + P_real, :], w_n[0:P_real, :])

        # software-pipelined emission: lookahead keeps every in-order queue
        # fed — phase2(si) runs while phase1(si+3) fills and phase3(si-1)
        # drains, so no engine waits on a 10-superblock serial prologue.
        LOOK = 2
        n_sb = len(sbs)
        for _rep in range(reps):
            for si in range(min(LOOK, n_sb)):
                emit_p1(si)
            for si in range(n_sb):
                # feed the DVE/Act queues with independent pre-pass and
                # drain work BEFORE this superblock's matmul-dependent ops,
                # so those queues never head-of-line block on the PE chain
                if si + LOOK < n_sb:
                    emit_p1(si + LOOK)
                if si >= 1:
                    emit_p3(si - 1)
                emit_p2(si)
            emit_p3(n_sb - 1)

    _split_waits(nc)
    return nc


def make_const_inputs() -> dict[str, np.ndarray]:
    off = np.full((F, 1), 64.0, dtype=np.float32)
    off[0:NPG] = 0.0
    off[64 : 64 + NPG] = float(K)
    return {
        "ident": np.eye(F, dtype=np.float32),
        "iota16": np.arange(2 * K, dtype=np.float32)[None, :] - off,
        "iotak": np.tile(np.arange(K, dtype=np.float32), (F, 1)),
    }


_NC_CACHE: dict[int, object] = {}
TRACE = False          # test harness sets True to collect an NTFF profile
LAST_RESULTS = None    # BassKernelResults of the most recent run


def _get_nc(g_core: int):
    if g_core not in _NC_CACHE:
        _NC_CACHE[g_core] = build_kernel(g_core)
    return _NC_CACHE[g_core]


def make_in_maps(inputs) -> list[dict[str, np.ndarray]]:
    x = np.ascontiguousarray(np.asarray(inputs["x"], dtype=np.float32))
    cls = np.ascontiguousarray(np.asarray(inputs["cls"], dtype=np.int32))
    w1 = np.asarray(inputs["W1"], dtype=np.float32)
    b1 = np.asarray(inputs["b1"], dtype=np.float32).reshape(F, 1)
    w2 = np.asarray(inputs["W2"], dtype=np.float32).reshape(F, 1)
    assert x.shape[0] == N_TOTAL, f"kernel hardcoded for N={N_TOTAL}"

    import ml_dtypes
    wpk = np.ascontiguousarray(
        np.concatenate([w1, b1, w2], axis=1).astype(ml_dtypes.bfloat16)
    )
    in_maps = []
    for core in range(N_CORES):
        lo, hi = core * N_CORE, (core + 1) * N_CORE
        in_maps.append(
            {
                "x": x[lo:hi],
                "cls": cls[lo:hi].reshape(G_CORE, NPG),
                "wpk": wpk,
            }
        )
    return in_maps


def kernel(**inputs) -> np.ndarray:
    nc = _get_nc(G_CORE)
    in_maps = make_in_maps(inputs)
    res = run_bass_kernel_spmd(nc, in_maps, list(range(N_CORES)), trace=TRACE)
    global LAST_RESULTS
    LAST_RESULTS = res
    outs = [res.results[c]["out"].reshape(N_CORE, 1) for c in range(N_CORES)]
    return np.ascontiguousarray(np.concatenate(outs, axis=0))


if __name__ == "__main__":
    ins = {
        "x": np.random.randn(N_TOTAL, F).astype(np.float32),
        "cls": np.random.randint(0, K, size=N_TOTAL).astype(np.int32),
        "batch": np.repeat(np.arange(G_TOTAL, dtype=np.int32), NPG),
        "W1": (np.random.randn(F, F) * 0.05).astype(np.float32),
        "b1": np.zeros(F, dtype=np.float32),
        "W2": (np.random.randn(F, 1) * 0.05).astype(np.float32),
        "b2": np.zeros(1, dtype=np.float32),
        "num_graphs": G_TOTAL,
        "num_clusters": K,
    }
    out = kernel(**ins)
    print(out.shape, out.dtype, out[:5, 0])

